# revision 1
# baseline (speedup 1.0000x reference)
"""ColorHistogramLoss Trainium2 kernel.

Computes mean(|soft_hist(pred) - soft_hist(target)|) for (4,3,512,512) f32
inputs, 64 Gaussian bins (sigma = 1/64).

Strategy (data-parallel over 8 NeuronCores, H-axis shard):
  - Each core receives 1/8 of the pixels of every (B,C) group of both
    tensors: 24 "streams" x 32768 pixels.
  - Key trick: Derivative_Erf activation = (2/sqrt(pi)) * exp(-x^2), so one
    ACT instruction per bin computes all Gaussian weights, with the free-dim
    reduction fused via accum_out. The 2/sqrt(pi) constant cancels in the
    histogram normalization (handled exactly on host).
  - Per-bin bias constants (-alpha * c_k) are packed as the first 64 columns
    of the input block (walrus limits sync-waits per instruction, so inputs
    ride along with the data DMA instead of extra parameter DMAs).
  - Host sums the per-partition partial histograms, normalizes, and takes
    the L1 mean (tiny: 8 x [128,128] floats).
"""

import math
import os
import sys

for _p in ("/opt/trn_rl_repo", "/root/.axon_site/_ro/trn_rl_repo"):
    if os.path.isdir(_p) and _p not in sys.path:
        sys.path.insert(0, _p)

import numpy as np

import concourse.bass as bass
import concourse.mybir as mybir

# Problem constants (hardcoded; kernel.py must be self-contained).
B, C, H, W = 4, 3, 512, 512
NB = 64                      # histogram bins
N_CORES = 8
SIGMA = 1.0 / NB
ALPHA = float(NB) / math.sqrt(2.0)   # t = ALPHA*x' - ALPHA*c_k ; w = exp(-t^2)
KAPPA = 2.0 / math.sqrt(math.pi)     # Derivative_Erf(x) = KAPPA * exp(-x^2)
EPS_CLIP = 1.0 - 1e-6

H_PER_CORE = H // N_CORES            # 64 rows
N_GROUPS = B * C                     # 12
N_STREAMS = 2 * N_GROUPS             # 24, interleaved (p0,t0,p1,t1,...) so a
                                     # pred group and its target twin share the
                                     # same layout -> bitwise-equal sums for
                                     # identical inputs
PIX_PER_STREAM = H_PER_CORE * W      # 32768

# SBUF layout: [128, NB + FB + FA]
#   cols [0, 64):        per-bin bias constants (same value down each column)
#   cols [64, 64+2048):  "block B" = streams 16..23, 16 partitions each, 2048 px
#   cols [2112, 6208):   "block A" = streams 0..15, 8 partitions each, 4096 px
# Bias + the small block B are DMA'd first so ACT starts after ~1 MB of DMA
# and a 2 us clamp; the 2 MB block-A DMA hides under B's 64 ACT instructions.
# Each block accumulates into its own hist columns; host adds partials.
FA = 4096
FB = 2048
W_IN = NB + FA + FB
# free-dim chunks in processing order: (col_start, width, accum col block)
_CHUNKS = ((NB, FB, 1), (NB + FB, FA, 0))
_NCH = len(_CHUNKS)

_CENTERS = (np.arange(NB, dtype=np.float64) + 0.5) / NB
_BIASES = (-ALPHA * _CENTERS).astype(np.float32)

_cached_callable = None


def _build_nc(n_iter: int = 1):
    """Build the bass program. n_iter > 1 replicates the whole pipeline
    (benchmarking only); the kernel output of the last iteration is DMA'd out
    each iteration identically."""
    nc = bass.Bass("TRN2", target_bir_lowering=False, debug=False)
    x_d = nc.dram_tensor("x", [128, W_IN], mybir.dt.float32, kind="ExternalInput").ap()
    hist_d = nc.dram_tensor(
        "hist", [128, _NCH * NB], mybir.dt.float32, kind="ExternalOutput"
    ).ap()

    with (
        nc.sbuf_tensor([128, W_IN], mybir.dt.float32) as xt,
        nc.sbuf_tensor([128, FA], mybir.dt.float32) as scratch,
        nc.sbuf_tensor([128, _NCH * NB], mybir.dt.float32) as hist,
        nc.semaphore() as dma_sem,
        nc.semaphore() as dve_sem,
        nc.semaphore() as act_sem,
        nc.Block() as block,
    ):

        @block.sync
        def _(sync):
            for i in range(n_iter):
                sync.wait_ge(act_sem, _NCH * i)
                sync.dma_start(xt[:, : NB + FB], x_d[:, : NB + FB]).then_inc(
                    dma_sem, 16
                )
                sync.dma_start(xt[:, NB + FB :], x_d[:, NB + FB :]).then_inc(
                    dma_sem, 16
                )
                sync.wait_ge(act_sem, _NCH * (i + 1))
                sync.dma_start(hist_d[:], hist[:]).then_inc(dma_sem, 16)
            sync.wait_ge(dma_sem, 16 * (_NCH + 1) * n_iter)

        @block.vector
        def _(vector):
            for i in range(n_iter):
                for j, (c0, w, _) in enumerate(_CHUNKS):
                    vector.wait_ge(dma_sem, 16 * ((_NCH + 1) * i + j + 1))
                    pix = xt[:, c0 : c0 + w]
                    # x' = clip(x*0.5 + 0.5, 0, 1 - 1e-6), matching the reference
                    vector.tensor_scalar(
                        pix, pix, 0.5, 0.5, mybir.AluOpType.mult, mybir.AluOpType.add
                    )
                    vector.tensor_scalar(
                        pix, pix, 0.0, EPS_CLIP, mybir.AluOpType.max, mybir.AluOpType.min
                    ).then_inc(dve_sem, 1)

        @block.scalar
        def _(scalar):
            # dummy activation at t=0: forces the Derivative_Erf table load
            # (~2.7 us) to happen during the first DMA instead of after it
            scalar.activation(
                scratch[:, :2],
                scratch[:, :2],
                mybir.ActivationFunctionType.Derivative_Erf,
                bias=scratch[:, 2:3],
                scale=ALPHA,
            )
            for i in range(n_iter):
                for j, (c0, w, blk) in enumerate(_CHUNKS):
                    scalar.wait_ge(dve_sem, _NCH * i + j + 1)
                    ins = None
                    for k in range(NB):
                        ins = scalar.activation(
                            scratch[:, :w],
                            xt[:, c0 : c0 + w],
                            mybir.ActivationFunctionType.Derivative_Erf,
                            bias=xt[:, k : k + 1],
                            scale=ALPHA,
                            accum_out=hist[:, blk * NB + k : blk * NB + k + 1],
                        )
                    ins.then_inc(act_sem, 1)

    return nc


def _pack_core_input(pred_c: np.ndarray, target_c: np.ndarray) -> np.ndarray:
    """pred_c/target_c: (B, C, H_PER_CORE, W) f32 -> [128, W_IN] input block."""
    streams = np.empty((N_STREAMS, PIX_PER_STREAM), dtype=np.float32)
    streams[0::2] = pred_c.reshape(N_GROUPS, PIX_PER_STREAM)
    streams[1::2] = target_c.reshape(N_GROUPS, PIX_PER_STREAM)
    block_a = streams[:16].reshape(128, FA)
    block_b = streams[16:].reshape(128, FB)
    bias_block = np.broadcast_to(_BIASES, (128, NB))
    return np.ascontiguousarray(
        np.concatenate([bias_block, block_b, block_a], axis=1), dtype=np.float32
    )


def _reduce_hists(results: list) -> np.ndarray:
    """Per-core [128, 128] partials -> (2, 12, 64) unnormalized histogram sums."""
    sums = np.zeros((N_STREAMS, NB), dtype=np.float64)
    for res in results:
        h = res["hist"].astype(np.float64)
        sums[:16] += h[:, :NB].reshape(16, 8, NB).sum(axis=1)
        sums[16:] += h[:, NB:].reshape(8, 16, NB).sum(axis=1)
    # stream s = 2g + (0 pred | 1 target)
    return np.stack([sums[0::2], sums[1::2]], axis=0)


def _finish(sums: np.ndarray) -> np.float32:
    """Normalize histograms exactly like the reference and take the L1 mean."""
    h = sums / KAPPA  # undo the Derivative_Erf constant
    hist = h / (h.sum(axis=-1, keepdims=True) + 1e-8)
    loss = np.abs(hist[0] - hist[1]).mean()
    return np.float32(loss)


def _get_callable():
    """Build the bass program once and wrap it in a persistent jitted
    shard_map callable over the 8-core mesh (re-tracing per call is ~1 s;
    this makes repeat kernel() calls cheap)."""
    global _cached_callable
    if _cached_callable is not None:
        return _cached_callable

    import jax
    from jax.sharding import Mesh, NamedSharding, PartitionSpec
    try:
        from jax import shard_map
    except ImportError:
        from jax.experimental.shard_map import shard_map
    from concourse.bass2jax import (
        _bass_exec_p,
        install_neuronx_cc_hook,
        partition_id_tensor,
    )

    nc = _build_nc()
    install_neuronx_cc_hook()

    pname = nc.partition_id_tensor.name if nc.partition_id_tensor else None
    in_names, out_names, out_avals = [], [], []
    for alloc in nc.m.functions[0].allocations:
        if not isinstance(alloc, mybir.MemoryLocationSet):
            continue
        name = alloc.memorylocations[0].name
        if alloc.kind == "ExternalInput" and name != pname:
            in_names.append(name)
        elif alloc.kind == "ExternalOutput":
            out_names.append(name)
            out_avals.append(
                jax.core.ShapedArray(
                    tuple(alloc.tensor_shape), mybir.dt.np(alloc.dtype)
                )
            )
    assert in_names == ["x"] and out_names == ["hist"]
    all_names = in_names + out_names + ([pname] if pname else [])

    def _body(*args):
        operands = list(args)
        if pname is not None:
            operands.append(partition_id_tensor())
        return tuple(
            _bass_exec_p.bind(
                *operands,
                out_avals=tuple(out_avals),
                in_names=tuple(all_names),
                out_names=tuple(out_names),
                lowering_input_output_aliases=(),
                sim_require_finite=True,
                sim_require_nnan=True,
                nc=nc,
            )
        )

    devices = jax.devices()[:N_CORES]
    mesh = Mesh(np.asarray(devices), ("core",))
    sm_kwargs = dict(
        mesh=mesh,
        in_specs=(PartitionSpec("core"),) * 2,
        out_specs=(PartitionSpec("core"),),
    )
    try:
        mapped = shard_map(_body, check_rep=False, **sm_kwargs)
    except TypeError:
        mapped = shard_map(_body, check_vma=False, **sm_kwargs)
    sharded = jax.jit(mapped, donate_argnums=(1,), keep_unused=True)
    sharding = NamedSharding(mesh, PartitionSpec("core"))
    out_shape = tuple(out_avals[0].shape)

    def run(xin_concat: np.ndarray) -> list:
        zeros = jax.device_put(
            np.zeros((N_CORES * out_shape[0], *out_shape[1:]), np.float32), sharding
        )
        (hist_out,) = sharded(jax.device_put(xin_concat, sharding), zeros)
        h = np.asarray(hist_out).reshape(N_CORES, *out_shape)
        return [{"hist": h[c]} for c in range(N_CORES)]

    _cached_callable = run
    return run


def _run(pred: np.ndarray, target: np.ndarray):
    run = _get_callable()

    pred = np.asarray(pred, dtype=np.float32)
    target = np.asarray(target, dtype=np.float32)

    blocks = []
    for c in range(N_CORES):
        rows = slice(c * H_PER_CORE, (c + 1) * H_PER_CORE)
        blocks.append(_pack_core_input(pred[:, :, rows, :], target[:, :, rows, :]))
    results = run(np.concatenate(blocks, axis=0))
    return _finish(_reduce_hists(results)), results


def kernel(pred: np.ndarray, target: np.ndarray) -> np.ndarray:
    loss, _ = _run(pred, target)
    return np.asarray(loss, dtype=np.float32)

